# revision 34
# baseline (speedup 1.0000x reference)
"""AttZip KV-cache eviction kernel for Trainium2 (8 NeuronCores, Bass/Tile).

Reference computation (B=1, H=32, S=4096, D=128, IMP=4, RECENT=512):
    imp_score   = attn.sum(q).mean(h)                      -> [B, S]
    keep_topk   = sort(top_k(imp_score[:, :S-RECENT], 4))
    keep_idx    = concat(keep_topk, arange(S-RECENT, S))   -> [B, 516]
    k_out/v_out = gather rows of k/v cache at keep_idx
    imp_out     = imp_score[keep_idx]
    counter_out = (S - keep_idx).astype(f32)

Sharding: tensor-parallel over the head axis (4 heads per core). Each core
streams its attn slice (268 MB) from HBM in 8 MB casting DMAs
(f32 -> float32r, which unlocks the PE's 1-cycle/row streaming rate), reduces
over (h, q) with ones-matmuls accumulating in PSUM, then one 16 KB AllReduce
combines the per-core head-sums. Top-4 selection runs on-device (DVE
max8/max_index + negate-and-sort trick); K/V rows are fetched with indirect
DMAs; the recent window is a plain DRAM->DRAM copy overlapped with the main
loop. Measured ~893 us on HW vs a ~835 us practical floor (the 268 MB stream
at the ~330 GB/s per-core rate observed when all 8 cores read HBM at once).
"""

import os
import sys

for _p in ("/opt/trn_rl_repo",):
    if _p not in sys.path and os.path.isdir(_p):
        sys.path.insert(0, _p)

import numpy as np

# Full-size problem config
FULL = dict(
    H=32, S=4096, D=128, IMP=4, RECENT=512, NCORES=8,
    TILE_Q=512, F32R=1, BUFS=2, DMA_SPLIT=0,
)


def build_program(
    H, S, D, IMP, RECENT, NCORES, TILE_Q,
    F32R=0, BUFS=3, DMA_SPLIT=0, MIX=0, DVE_D=1, DVE_M=2,
):
    """Build the (identical-per-core) Bass program for the given sizes."""
    import concourse.bacc as bacc
    import concourse.bass as bass
    import concourse.mybir as mybir
    import concourse.tile as tile

    f32 = mybir.dt.float32
    i32 = mybir.dt.int32
    u32 = mybir.dt.uint32
    Alu = mybir.AluOpType

    HPC = H // NCORES          # heads per core
    OLD = S - RECENT           # prefix length eligible for top-k
    CACHE = IMP + RECENT
    NBLK = TILE_Q // 128       # 128-row blocks per attn DMA chunk
    NCHUNK = S // TILE_Q       # chunks per head along q
    NJ = S // 512              # 512-wide matmul column chunks
    NGAT = HPC * IMP           # gathered K/V rows per core

    assert TILE_Q % 128 == 0 and S % TILE_Q == 0 and S % 512 == 0

    nc = bacc.Bacc(
        "TRN2",
        target_bir_lowering=False,
        debug=False,
        enable_asserts=False,
        num_devices=NCORES,
    )

    attn = nc.dram_tensor("attn", [HPC, S, S], f32, kind="ExternalInput").ap()
    kc = nc.dram_tensor("kc", [HPC, S, D], f32, kind="ExternalInput").ap()
    vc = nc.dram_tensor("vc", [HPC, S, D], f32, kind="ExternalInput").ap()
    # headoff[h*IMP + j] = h * S  (row offset of head h in the flattened cache)
    headoff = nc.dram_tensor("headoff", [NGAT, 1], f32, kind="ExternalInput").ap()
    # cnt_rec[i] = S - (OLD + i): the static recent part of counter_out
    cnt_rec = nc.dram_tensor("cnt_rec", [1, RECENT], f32, kind="ExternalInput").ap()

    k_out = nc.dram_tensor("k_out", [HPC, CACHE, D], f32, kind="ExternalOutput").ap()
    v_out = nc.dram_tensor("v_out", [HPC, CACHE, D], f32, kind="ExternalOutput").ap()
    imp_out = nc.dram_tensor("imp_out", [1, CACHE], f32, kind="ExternalOutput").ap()
    cnt_out = nc.dram_tensor("cnt_out", [1, CACHE], f32, kind="ExternalOutput").ap()

    with tile.TileContext(nc) as tc:
        with (
            tc.tile_pool(name="stream", bufs=BUFS) as stream,
            tc.tile_pool(name="small", bufs=1) as small,
            tc.tile_pool(name="psum", bufs=1, space="PSUM") as psum_pool,
            tc.tile_pool(name="dram", bufs=1, space="DRAM") as dram,
        ):
            # ---- static copies (independent of everything; overlap the main loop)
            nc.sync.dma_start(out=k_out[:, IMP:, :], in_=kc[:, OLD:, :])
            nc.sync.dma_start(out=v_out[:, IMP:, :], in_=vc[:, OLD:, :])
            nc.sync.dma_start(out=cnt_out[:, IMP:], in_=cnt_rec[:, :])

            # ---- phase 1: head-sum reduction over (h, q) via PE ones-matmul
            ones = small.tile([128, 1], f32)
            nc.vector.memset(ones[:], 1.0)
            acc = psum_pool.tile([1, S], f32)

            # float32r streams the moving operand at 1 cycle/row (vs 4 for
            # float32) when the free dim is >=256. The verifier requires the
            # producer of an f32r matmul input to round to f32r, which the
            # SWDGE casting DMA (f32 DRAM -> f32r SBUF tile) satisfies.
            if MIX:
                F32R = 0
            mm_dt = mybir.dt.float32r if F32R else f32
            ones_mm = ones[:].bitcast(mm_dt)
            # MIX mode: exact-f32 reduction split between PE (psum matmul
            # accumulate) and DVE (SBUF tensor_tensor accumulate); block g
            # goes to DVE iff g % DVE_M < DVE_D.
            acc_sb = small.tile([128, S], f32, name="acc_sb") if MIX else None
            n_dve_blocks = (
                sum(1 for g in range(HPC * (S // 128)) if g % DVE_M < DVE_D)
                if MIX
                else 0
            )

            # PE warmup: depends only on `ones`, so the real first matmul
            # carries a single sync wait (the attn-tile DMA). Its output is
            # erased by the first start=True matmul on the same region.
            nc.tensor.matmul(
                out=acc[0:1, 0:1],
                lhsT=ones[:],
                rhs=ones[:, 0:1],
                start=True,
                stop=True,
                skip_group_check=True,
            )

            # chunk schedule: small leading chunks get the PE started early;
            # small trailing chunks let the PE (and the psum drain) finish
            # right behind the last DMA byte.
            chunks = []  # (head, q0, rows)
            for h in range(HPC):
                front = [128, 128, 256] if h == 0 else []
                back = [256, 128, 128] if h == HPC - 1 else []
                front = [r for r in front if r < TILE_Q]
                back = [r for r in back if r < TILE_Q]
                body = S - sum(front) - sum(back)
                if body < 0 or body % 128:
                    back = []
                    body = S - sum(front)
                if body < 0 or body % 128:
                    front = []
                    body = S
                sizes = (
                    front
                    + [TILE_Q] * (body // TILE_Q)
                    + [128] * ((body % TILE_Q) // 128)
                    + back
                )
                q0 = 0
                for r in sizes:
                    chunks.append((h, q0, r))
                    q0 += r
                assert q0 == S

            gidx = 0          # global 128-row block counter
            pe_blocks = 0     # PE streaming blocks emitted so far
            dve_blocks = 0    # DVE blocks emitted so far
            n_pe_blocks = HPC * (S // 128) - n_dve_blocks
            for h, q0, rows in chunks:
                    nblk = rows // 128
                    t = stream.tile([128, nblk, S], mm_dt, tag="attn_tile")
                    src = attn[h, q0 : q0 + rows, :].rearrange(
                        "(b p) k -> p b k", p=128
                    )
                    dma_eng = nc.gpsimd if F32R else nc.sync
                    if DMA_SPLIT:
                        # one DMA per 128-row block: more queue-lane overlap
                        for b in range(nblk):
                            dma_eng.dma_start(
                                out=t[:, b : b + 1, :], in_=src[:, b : b + 1, :]
                            )
                    else:
                        dma_eng.dma_start(out=t[:], in_=src)
                    for b in range(nblk):
                        to_dve = MIX and gidx % DVE_M < DVE_D
                        if to_dve:
                            if dve_blocks == 0:
                                nc.vector.tensor_copy(acc_sb[:], t[:, b, :])
                            else:
                                nc.vector.tensor_add(
                                    acc_sb[:], acc_sb[:], t[:, b, :]
                                )
                            dve_blocks += 1
                        else:
                            first = pe_blocks == 0
                            last = (
                                pe_blocks == n_pe_blocks - 1 and n_dve_blocks == 0
                            )
                            for j in range(NJ):
                                nc.tensor.matmul(
                                    out=acc[0:1, j * 512 : (j + 1) * 512],
                                    lhsT=ones_mm,
                                    rhs=t[:, b, j * 512 : (j + 1) * 512],
                                    start=first,
                                    stop=last,
                                    skip_group_check=True,
                                )
                            pe_blocks += 1
                        gidx += 1

            if n_dve_blocks:
                # fold the DVE accumulator into PSUM: per-bank f32 colsum
                for j in range(NJ):
                    nc.tensor.matmul(
                        out=acc[0:1, j * 512 : (j + 1) * 512],
                        lhsT=ones[:],
                        rhs=acc_sb[:, j * 512 : (j + 1) * 512],
                        start=(n_pe_blocks == 0),
                        stop=True,
                        skip_group_check=True,
                    )

            # Drain PSUM per bank so bank j's copy+DMA overlaps the remaining
            # matmuls of banks j+1..NJ-1 (bank j's last write is early in the
            # final chunk).
            headsum = small.tile([1, S], f32)
            bounce_in = dram.tile([1, S], f32)
            bounce_out = dram.tile([S, 1], f32)  # [S,1] so it can be a gather source
            for j in range(NJ):
                sl = slice(j * 512, (j + 1) * 512)
                nc.vector.tensor_copy(headsum[:, sl], acc[0:1, sl])
                nc.sync.dma_start(bounce_in[:, sl], headsum[:, sl])
            nc.gpsimd.collective_compute(
                "AllReduce",
                Alu.add,
                replica_groups=[list(range(NCORES))],
                ins=[bounce_in.opt()],
                outs=[bounce_out.opt()],
            )
            sum_sb = small.tile([1, S], f32)
            nc.sync.dma_start(sum_sb[:], bounce_out.rearrange("s one -> one s"))

            # ---- phase 3: top-IMP of sum_sb[0:OLD], indices sorted ascending
            max8 = small.tile([1, 8], f32)
            nc.vector.max(out=max8[:], in_=sum_sb[:, 0:OLD])
            idx8 = small.tile([1, 8], u32)
            nc.vector.max_index(out=idx8[:], in_max=max8[:], in_values=sum_sb[:, 0:OLD])
            idx8f = small.tile([1, 8], f32)
            nc.vector.tensor_copy(idx8f[:], idx8[:])
            # negate the top-IMP indices, pad with -BIG, then max8 sorts them
            # descending -> negated ascending index order
            negidx = small.tile([1, 8], f32)
            nc.vector.memset(negidx[:], -1.0e9)
            nc.vector.tensor_scalar(
                negidx[:, 0:IMP], idx8f[:, 0:IMP], -1.0, None, op0=Alu.mult
            )
            sortneg = small.tile([1, 8], f32)
            nc.vector.max(out=sortneg[:], in_=negidx[:])
            keep4f = small.tile([1, IMP], f32)
            nc.vector.tensor_scalar(
                keep4f[:], sortneg[:, 0:IMP], -1.0, None, op0=Alu.mult
            )
            # counter_out[0:IMP] = S - keep_idx
            cnt4 = small.tile([1, IMP], f32)
            nc.vector.tensor_scalar(
                cnt4[:], keep4f[:], -1.0, float(S), op0=Alu.mult, op1=Alu.add
            )
            nc.sync.dma_start(cnt_out[:, 0:IMP], cnt4[:])

            # ---- phase 4: build per-partition gather indices [NGAT, 1]
            idx_dram = dram.tile([1, IMP], f32)
            nc.sync.dma_start(idx_dram[:], keep4f[:])
            idx16f = small.tile([NGAT, 1], f32)
            nc.sync.dma_start(
                idx16f[:], idx_dram[0:1, :].to_broadcast([HPC, IMP])
            )
            hoff = small.tile([NGAT, 1], f32)
            nc.sync.dma_start(hoff[:], headoff[:, :])
            nc.vector.tensor_scalar(
                idx16f[:], idx16f[:], hoff[:, 0:1], None, op0=Alu.add
            )
            idx16 = small.tile([NGAT, 1], i32)
            nc.vector.tensor_copy(idx16[:], idx16f[:])

            # ---- phase 5: gathers
            kgat = small.tile([NGAT, D], f32)
            nc.gpsimd.indirect_dma_start(
                out=kgat[:],
                out_offset=None,
                in_=kc.rearrange("h s d -> (h s) d"),
                in_offset=bass.IndirectOffsetOnAxis(ap=idx16[:, 0:1], axis=0),
            )
            nc.sync.dma_start(k_out[:, 0:IMP, :], kgat[:])
            vgat = small.tile([NGAT, D], f32)
            nc.gpsimd.indirect_dma_start(
                out=vgat[:],
                out_offset=None,
                in_=vc.rearrange("h s d -> (h s) d"),
                in_offset=bass.IndirectOffsetOnAxis(ap=idx16[:, 0:1], axis=0),
            )
            nc.sync.dma_start(v_out[:, 0:IMP, :], vgat[:])

            # imp_out[0:IMP]: gather the selected head-sums, scale by 1/H
            imp4 = small.tile([IMP, 1], f32)
            nc.gpsimd.indirect_dma_start(
                out=imp4[:],
                out_offset=None,
                in_=bounce_out[:],
                in_offset=bass.IndirectOffsetOnAxis(ap=idx16[0:IMP, 0:1], axis=0),
            )
            nc.vector.tensor_scalar(imp4[:], imp4[:], 1.0 / H, None, op0=Alu.mult)
            nc.sync.dma_start(imp_out[:, 0:IMP], imp4[:])
            # imp_out[IMP:]: recent slice of the head-mean
            imp_rec = small.tile([1, RECENT], f32)
            nc.vector.tensor_scalar(
                imp_rec[:], sum_sb[:, OLD:S], 1.0 / H, None, op0=Alu.mult
            )
            nc.sync.dma_start(imp_out[:, IMP:], imp_rec[:])

    nc.compile()
    return nc


def make_host_constants(H, S, D, IMP, RECENT, NCORES, TILE_Q, **_):
    HPC = H // NCORES
    OLD = S - RECENT
    headoff = np.repeat(
        (np.arange(HPC, dtype=np.float32) * S), IMP
    ).reshape(HPC * IMP, 1)
    cnt_rec = (S - np.arange(OLD, S, dtype=np.float32)).reshape(1, RECENT)
    return headoff, cnt_rec


def make_in_maps(k_cache, v_cache, attn_score_cache, cfg):
    H, S, NCORES = cfg["H"], cfg["S"], cfg["NCORES"]
    HPC = H // NCORES
    headoff, cnt_rec = make_host_constants(**cfg)
    k_cache = np.asarray(k_cache)
    v_cache = np.asarray(v_cache)
    attn_score_cache = np.asarray(attn_score_cache)
    in_maps = []
    for c in range(NCORES):
        hs = slice(c * HPC, (c + 1) * HPC)
        in_maps.append(
            {
                "attn": np.ascontiguousarray(attn_score_cache[0, hs]),
                "kc": np.ascontiguousarray(k_cache[0, hs]),
                "vc": np.ascontiguousarray(v_cache[0, hs]),
                "headoff": headoff,
                "cnt_rec": cnt_rec,
            }
        )
    return in_maps


def assemble_outputs(results, cfg):
    NCORES = cfg["NCORES"]
    k_out = np.concatenate([results[c]["k_out"] for c in range(NCORES)], axis=0)[None]
    v_out = np.concatenate([results[c]["v_out"] for c in range(NCORES)], axis=0)[None]
    imp_out = results[0]["imp_out"].reshape(1, -1)
    cnt_out = results[0]["cnt_out"].reshape(1, -1)
    return k_out, v_out, imp_out, cnt_out


_PROGRAM_CACHE = {}


def _get_program(cfg_key):
    if cfg_key not in _PROGRAM_CACHE:
        _PROGRAM_CACHE[cfg_key] = build_program(**dict(cfg_key))
    return _PROGRAM_CACHE[cfg_key]


def run(k_cache, v_cache, attn_score_cache, trace=False, cfg=None):
    """Run on hardware; returns (outputs_tuple, BassKernelResults)."""
    from concourse import bass_utils

    cfg = cfg or FULL
    nc = _get_program(tuple(sorted(cfg.items())))
    in_maps = make_in_maps(k_cache, v_cache, attn_score_cache, cfg)
    res = bass_utils.run_bass_kernel_spmd(
        nc, in_maps, core_ids=list(range(cfg["NCORES"])), trace=trace
    )
    return assemble_outputs(res.results, cfg), res


def kernel(k_cache, v_cache, attn_score_cache):
    outs, _ = run(k_cache, v_cache, attn_score_cache, trace=False)
    return outs


# revision 43
# speedup vs baseline: 1.0523x; 1.0523x over previous
"""AttZip KV-cache eviction kernel for Trainium2 (8 NeuronCores, Bass/Tile).

Reference computation (B=1, H=32, S=4096, D=128, IMP=4, RECENT=512):
    imp_score   = attn.sum(q).mean(h)                      -> [B, S]
    keep_topk   = sort(top_k(imp_score[:, :S-RECENT], 4))
    keep_idx    = concat(keep_topk, arange(S-RECENT, S))   -> [B, 516]
    k_out/v_out = gather rows of k/v cache at keep_idx
    imp_out     = imp_score[keep_idx]
    counter_out = (S - keep_idx).astype(f32)

Sharding: tensor-parallel over the head axis (4 heads per core). Each core
streams its attn slice (268 MB) from HBM in 8 MB casting DMAs
(f32 -> float32r, which unlocks the PE's 1-cycle/row streaming rate), reduces
over (h, q) with ones-matmuls accumulating in PSUM, then one 16 KB AllReduce
combines the per-core head-sums. Top-4 selection runs on-device (DVE
max8/max_index + negate-and-sort trick); K/V rows are fetched with indirect
DMAs; the recent window is a plain DRAM->DRAM copy overlapped with the main
loop. Measured ~893 us on HW vs a ~835 us practical floor (the 268 MB stream
at the ~330 GB/s per-core rate observed when all 8 cores read HBM at once).
"""

import os
import sys

for _p in ("/opt/trn_rl_repo",):
    if _p not in sys.path and os.path.isdir(_p):
        sys.path.insert(0, _p)

import numpy as np

# Full-size problem config
FULL = dict(
    H=32, S=4096, D=128, IMP=4, RECENT=512, NCORES=8,
    TILE_Q=512, F32R=1, BUFS=2, DMA_SPLIT=0, PMAJOR=1,
)


def build_program(
    H, S, D, IMP, RECENT, NCORES, TILE_Q,
    F32R=0, BUFS=3, DMA_SPLIT=0, MIX=0, DVE_D=1, DVE_M=2, PMAJOR=0,
):
    """Build the (identical-per-core) Bass program for the given sizes."""
    import concourse.bacc as bacc
    import concourse.bass as bass
    import concourse.mybir as mybir
    import concourse.tile as tile

    f32 = mybir.dt.float32
    i32 = mybir.dt.int32
    u32 = mybir.dt.uint32
    Alu = mybir.AluOpType

    HPC = H // NCORES          # heads per core
    OLD = S - RECENT           # prefix length eligible for top-k
    CACHE = IMP + RECENT
    NBLK = TILE_Q // 128       # 128-row blocks per attn DMA chunk
    NCHUNK = S // TILE_Q       # chunks per head along q
    NJ = S // 512              # 512-wide matmul column chunks
    NGAT = HPC * IMP           # gathered K/V rows per core

    assert TILE_Q % 128 == 0 and S % TILE_Q == 0 and S % 512 == 0

    nc = bacc.Bacc(
        "TRN2",
        target_bir_lowering=False,
        debug=False,
        enable_asserts=False,
        num_devices=NCORES,
    )

    attn = nc.dram_tensor("attn", [HPC, S, S], f32, kind="ExternalInput").ap()
    kc = nc.dram_tensor("kc", [HPC, S, D], f32, kind="ExternalInput").ap()
    vc = nc.dram_tensor("vc", [HPC, S, D], f32, kind="ExternalInput").ap()
    # headoff[h*IMP + j] = h * S  (row offset of head h in the flattened cache)
    headoff = nc.dram_tensor("headoff", [NGAT, 1], f32, kind="ExternalInput").ap()
    # cnt_rec[i] = S - (OLD + i): the static recent part of counter_out
    cnt_rec = nc.dram_tensor("cnt_rec", [1, RECENT], f32, kind="ExternalInput").ap()

    k_out = nc.dram_tensor("k_out", [HPC, CACHE, D], f32, kind="ExternalOutput").ap()
    v_out = nc.dram_tensor("v_out", [HPC, CACHE, D], f32, kind="ExternalOutput").ap()
    imp_out = nc.dram_tensor("imp_out", [1, CACHE], f32, kind="ExternalOutput").ap()
    cnt_out = nc.dram_tensor("cnt_out", [1, CACHE], f32, kind="ExternalOutput").ap()

    with tile.TileContext(nc) as tc:
        with (
            tc.tile_pool(name="stream", bufs=BUFS) as stream,
            tc.tile_pool(name="small", bufs=1) as small,
            tc.tile_pool(name="psum", bufs=1, space="PSUM") as psum_pool,
            tc.tile_pool(name="dram", bufs=1, space="DRAM") as dram,
        ):
            # ---- static copies (independent of everything; overlap the main loop)
            nc.sync.dma_start(out=k_out[:, IMP:, :], in_=kc[:, OLD:, :])
            nc.sync.dma_start(out=v_out[:, IMP:, :], in_=vc[:, OLD:, :])
            nc.sync.dma_start(out=cnt_out[:, IMP:], in_=cnt_rec[:, :])

            # ---- phase 1: head-sum reduction over (h, q) via PE ones-matmul
            ones = small.tile([128, 1], f32)
            nc.vector.memset(ones[:], 1.0)
            acc = psum_pool.tile([1, S], f32, tag="accbank")

            # hoisted off the post-collective critical path
            negidx = small.tile([1, 8], f32)
            nc.vector.memset(negidx[:], -1.0e9)
            hoff = small.tile([NGAT, 1], f32)
            nc.sync.dma_start(hoff[:], headoff[:, :])

            # float32r streams the moving operand at 1 cycle/row (vs 4 for
            # float32) when the free dim is >=256. The verifier requires the
            # producer of an f32r matmul input to round to f32r, which the
            # SWDGE casting DMA (f32 DRAM -> f32r SBUF tile) satisfies.
            if MIX:
                F32R = 0
            mm_dt = mybir.dt.float32r if F32R else f32
            ones_mm = ones[:].bitcast(mm_dt)
            # MIX mode: exact-f32 reduction split between PE (psum matmul
            # accumulate) and DVE (SBUF tensor_tensor accumulate); block g
            # goes to DVE iff g % DVE_M < DVE_D.
            acc_sb = small.tile([128, S], f32, name="acc_sb") if MIX else None
            n_dve_blocks = (
                sum(1 for g in range(HPC * (S // 128)) if g % DVE_M < DVE_D)
                if MIX
                else 0
            )

            # PE warmup: depends only on `ones`, so the real first matmul
            # carries a single sync wait (the attn-tile DMA). Its output is
            # erased by the first start=True matmul on the same region.
            nc.tensor.matmul(
                out=acc[0:1, 0:1],
                lhsT=ones[:],
                rhs=ones[:, 0:1],
                start=True,
                stop=True,
                skip_group_check=True,
            )

            # chunk schedule: small leading chunks get the PE started early;
            # small trailing chunks let the PE (and the psum drain) finish
            # right behind the last DMA byte.
            chunks = []  # (head, q0, rows)
            for h in range(HPC):
                front = [128, 128, 256] if h == 0 else []
                back = [256, 128, 128] if h == HPC - 1 else []
                front = [r for r in front if r < TILE_Q]
                back = [r for r in back if r < TILE_Q]
                body = S - sum(front) - sum(back)
                if body < 0 or body % 128:
                    back = []
                    body = S - sum(front)
                if body < 0 or body % 128:
                    front = []
                    body = S
                sizes = (
                    front
                    + [TILE_Q] * (body // TILE_Q)
                    + [128] * ((body % TILE_Q) // 128)
                    + back
                )
                q0 = 0
                for r in sizes:
                    chunks.append((h, q0, r))
                    q0 += r
                assert q0 == S

            gidx = 0          # global 128-row block counter
            pe_blocks = 0     # PE streaming blocks emitted so far
            dve_blocks = 0    # DVE blocks emitted so far
            n_pe_blocks = HPC * (S // 128) - n_dve_blocks
            for h, q0, rows in chunks:
                    nblk = rows // 128
                    t = stream.tile([128, nblk, S], mm_dt, tag="attn_tile")
                    # Row->(partition, block) mapping. p-major gives each
                    # partition one contiguous nblk*16KB DRAM segment (bigger
                    # DMA descriptors); the (h,q)-sum is invariant to the
                    # permutation since we reduce over both p and b.
                    if PMAJOR:
                        src = attn[h, q0 : q0 + rows, :].rearrange(
                            "(p b) k -> p b k", p=128
                        )
                    else:
                        src = attn[h, q0 : q0 + rows, :].rearrange(
                            "(b p) k -> p b k", p=128
                        )
                    dma_eng = nc.gpsimd if F32R else nc.sync
                    if DMA_SPLIT:
                        # one DMA per 128-row block: more queue-lane overlap
                        for b in range(nblk):
                            dma_eng.dma_start(
                                out=t[:, b : b + 1, :], in_=src[:, b : b + 1, :]
                            )
                    else:
                        dma_eng.dma_start(out=t[:], in_=src)
                    for b in range(nblk):
                        to_dve = MIX and gidx % DVE_M < DVE_D
                        if to_dve:
                            if dve_blocks == 0:
                                nc.vector.tensor_copy(acc_sb[:], t[:, b, :])
                            else:
                                nc.vector.tensor_add(
                                    acc_sb[:], acc_sb[:], t[:, b, :]
                                )
                            dve_blocks += 1
                        else:
                            first = pe_blocks == 0
                            last = (
                                pe_blocks == n_pe_blocks - 1 and n_dve_blocks == 0
                            )
                            for j in range(NJ):
                                nc.tensor.matmul(
                                    out=acc[0:1, j * 512 : (j + 1) * 512],
                                    lhsT=ones_mm,
                                    rhs=t[:, b, j * 512 : (j + 1) * 512],
                                    start=first,
                                    stop=last,
                                    skip_group_check=True,
                                )
                            pe_blocks += 1
                        gidx += 1

            if n_dve_blocks:
                # fold the DVE accumulator into PSUM: per-bank f32 colsum
                for j in range(NJ):
                    nc.tensor.matmul(
                        out=acc[0:1, j * 512 : (j + 1) * 512],
                        lhsT=ones[:],
                        rhs=acc_sb[:, j * 512 : (j + 1) * 512],
                        start=(n_pe_blocks == 0),
                        stop=True,
                        skip_group_check=True,
                    )

            # Drain PSUM per bank so bank j's copy+DMA overlaps the remaining
            # matmuls of banks j+1..NJ-1 (bank j's last write is early in the
            # final chunk).
            headsum = small.tile([1, S], f32)
            bounce_in = dram.tile([1, S], f32)
            bounce_out = dram.tile([S, 1], f32)  # [S,1] so it can be a gather source
            for j in range(NJ):
                sl = slice(j * 512, (j + 1) * 512)
                nc.vector.tensor_copy(headsum[:, sl], acc[0:1, sl])
                nc.sync.dma_start(bounce_in[:, sl], headsum[:, sl])
            nc.gpsimd.collective_compute(
                "AllReduce",
                Alu.add,
                replica_groups=[list(range(NCORES))],
                ins=[bounce_in.opt()],
                outs=[bounce_out.opt()],
            )
            sum_sb = small.tile([1, S], f32)
            nc.sync.dma_start(sum_sb[:], bounce_out.rearrange("s one -> one s"))

            # ---- phase 3: top-IMP of sum_sb[0:OLD], indices sorted ascending
            max8 = small.tile([1, 8], f32)
            nc.vector.max(out=max8[:], in_=sum_sb[:, 0:OLD])
            idx8 = small.tile([1, 8], u32)
            nc.vector.max_index(out=idx8[:], in_max=max8[:], in_values=sum_sb[:, 0:OLD])
            # negate the top-IMP indices (u32 read, fp32 internal math is
            # exact), pad with -BIG (hoisted memset), then max8 sorts them
            # descending -> negated ascending index order
            nc.vector.tensor_scalar(
                negidx[:, 0:IMP], idx8[:, 0:IMP], -1.0, None, op0=Alu.mult
            )
            sortneg = small.tile([1, 8], f32)
            nc.vector.max(out=sortneg[:], in_=negidx[:])

            # ---- phase 4: build per-partition gather indices [NGAT, 1].
            # Un-negate straight into keep16f, tile IMP->NGAT in the free dim
            # (doubling copies), then a PE transpose (identity = ones[0:1,0:1])
            # moves it onto NGAT partitions -- no DRAM round-trips on the
            # critical path.
            keep16f = small.tile([1, NGAT], f32)
            nc.vector.tensor_scalar(
                keep16f[:, 0:IMP], sortneg[:, 0:IMP], -1.0, None, op0=Alu.mult
            )
            filled = IMP
            while filled < NGAT:
                n = min(filled, NGAT - filled)
                nc.vector.tensor_copy(
                    keep16f[:, filled : filled + n], keep16f[:, 0:n]
                )
                filled += n
            psum_t = psum_pool.tile([NGAT, 1], f32, tag="accbank")
            nc.tensor.transpose(
                out=psum_t[:], in_=keep16f[:], identity=ones[0:1, 0:1]
            )
            idx16f = small.tile([NGAT, 1], f32)
            nc.vector.tensor_scalar(
                idx16f[:], psum_t[:], hoff[:, 0:1], None, op0=Alu.add
            )
            idx16 = small.tile([NGAT, 1], i32)
            nc.vector.tensor_copy(idx16[:], idx16f[:])

            # ---- phase 5: gathers
            kgat = small.tile([NGAT, D], f32)
            nc.gpsimd.indirect_dma_start(
                out=kgat[:],
                out_offset=None,
                in_=kc.rearrange("h s d -> (h s) d"),
                in_offset=bass.IndirectOffsetOnAxis(ap=idx16[:, 0:1], axis=0),
            )
            nc.sync.dma_start(k_out[:, 0:IMP, :], kgat[:])
            vgat = small.tile([NGAT, D], f32)
            nc.gpsimd.indirect_dma_start(
                out=vgat[:],
                out_offset=None,
                in_=vc.rearrange("h s d -> (h s) d"),
                in_offset=bass.IndirectOffsetOnAxis(ap=idx16[:, 0:1], axis=0),
            )
            nc.sync.dma_start(v_out[:, 0:IMP, :], vgat[:])

            # imp_out[0:IMP]: gather the selected head-sums, scale by 1/H
            imp4 = small.tile([IMP, 1], f32)
            nc.gpsimd.indirect_dma_start(
                out=imp4[:],
                out_offset=None,
                in_=bounce_out[:],
                in_offset=bass.IndirectOffsetOnAxis(ap=idx16[0:IMP, 0:1], axis=0),
            )
            nc.vector.tensor_scalar(imp4[:], imp4[:], 1.0 / H, None, op0=Alu.mult)
            nc.sync.dma_start(imp_out[:, 0:IMP], imp4[:])
            # imp_out[IMP:]: recent slice of the head-mean
            imp_rec = small.tile([1, RECENT], f32)
            nc.vector.tensor_scalar(
                imp_rec[:], sum_sb[:, OLD:S], 1.0 / H, None, op0=Alu.mult
            )
            nc.sync.dma_start(imp_out[:, IMP:], imp_rec[:])
            # counter_out[0:IMP] = S - keep_idx (off the gather critical path)
            cnt4 = small.tile([1, IMP], f32)
            nc.vector.tensor_scalar(
                cnt4[:], keep16f[:, 0:IMP], -1.0, float(S), op0=Alu.mult, op1=Alu.add
            )
            nc.sync.dma_start(cnt_out[:, 0:IMP], cnt4[:])

    nc.compile()
    return nc


def make_host_constants(H, S, D, IMP, RECENT, NCORES, TILE_Q, **_):
    HPC = H // NCORES
    OLD = S - RECENT
    headoff = np.repeat(
        (np.arange(HPC, dtype=np.float32) * S), IMP
    ).reshape(HPC * IMP, 1)
    cnt_rec = (S - np.arange(OLD, S, dtype=np.float32)).reshape(1, RECENT)
    return headoff, cnt_rec


def make_in_maps(k_cache, v_cache, attn_score_cache, cfg):
    H, S, NCORES = cfg["H"], cfg["S"], cfg["NCORES"]
    HPC = H // NCORES
    headoff, cnt_rec = make_host_constants(**cfg)
    k_cache = np.asarray(k_cache)
    v_cache = np.asarray(v_cache)
    attn_score_cache = np.asarray(attn_score_cache)
    in_maps = []
    for c in range(NCORES):
        hs = slice(c * HPC, (c + 1) * HPC)
        in_maps.append(
            {
                "attn": np.ascontiguousarray(attn_score_cache[0, hs]),
                "kc": np.ascontiguousarray(k_cache[0, hs]),
                "vc": np.ascontiguousarray(v_cache[0, hs]),
                "headoff": headoff,
                "cnt_rec": cnt_rec,
            }
        )
    return in_maps


def assemble_outputs(results, cfg):
    NCORES = cfg["NCORES"]
    k_out = np.concatenate([results[c]["k_out"] for c in range(NCORES)], axis=0)[None]
    v_out = np.concatenate([results[c]["v_out"] for c in range(NCORES)], axis=0)[None]
    imp_out = results[0]["imp_out"].reshape(1, -1)
    cnt_out = results[0]["cnt_out"].reshape(1, -1)
    return k_out, v_out, imp_out, cnt_out


_PROGRAM_CACHE = {}


def _get_program(cfg_key):
    if cfg_key not in _PROGRAM_CACHE:
        _PROGRAM_CACHE[cfg_key] = build_program(**dict(cfg_key))
    return _PROGRAM_CACHE[cfg_key]


def run(k_cache, v_cache, attn_score_cache, trace=False, cfg=None):
    """Run on hardware; returns (outputs_tuple, BassKernelResults)."""
    from concourse import bass_utils

    cfg = cfg or FULL
    nc = _get_program(tuple(sorted(cfg.items())))
    in_maps = make_in_maps(k_cache, v_cache, attn_score_cache, cfg)
    res = bass_utils.run_bass_kernel_spmd(
        nc, in_maps, core_ids=list(range(cfg["NCORES"])), trace=trace
    )
    return assemble_outputs(res.results, cfg), res


def kernel(k_cache, v_cache, attn_score_cache):
    outs, _ = run(k_cache, v_cache, attn_score_cache, trace=False)
    return outs
